# revision 1
# baseline (speedup 1.0000x reference)
"""Multi-head graph attention message passing on 8 Trainium2 cores.

Strategy (graph/data parallel, per the dst-sharding scheme):
  - Nodes sharded by dst across 8 cores (12500 each). Each core owns the
    wV rows for its dst range; segment_sum is local via hardware
    dma_scatter_add (CCE accumulate in the SDMA datapath).
  - Q/K/V projections: small weights replicated; every core computes the
    full K,V tables (replicated compute, no cross-core traffic) and the
    Q table for its own node range only. K,V stored interleaved per node
    row [K|V] so one dma_gather per edge fetches both.
  - Edges routed by dst partition on host; within a core, grouped by src
    chunk (4 chunks) so dma_gather int16 indices stay in range.
"""

import numpy as np

import concourse.bacc as bacc
import concourse.mybir as mybir
import concourse.tile as tile
from concourse.bass_utils import run_bass_kernel_spmd

F32 = mybir.dt.float32
I16 = mybir.dt.int16


class Cfg:
    n_nodes = 100000
    n_edges = 1600000
    in_dim = 128
    heads = 8
    hdim = 16
    hid = 128          # heads * hdim
    n_cores = 8
    n_chunks = 4       # src chunks for int16 gather indices
    batch = 1024       # edges per device batch (dma_gather caps near 1024 idxs/call)
    proj_tile = 512    # nodes per projection DMA group
    kv_bf16 = False    # store K,V tables in bf16 (halves gather traffic)

    def __init__(self, **kw):
        for k, v in kw.items():
            setattr(self, k, v)
        assert self.n_nodes % self.n_cores == 0
        self.own = self.n_nodes // self.n_cores
        # padded full node count: multiple of proj_tile and n_chunks
        m = self.proj_tile * self.n_chunks
        self.np_pad = -(-self.n_nodes // m) * m
        self.cr = self.np_pad // self.n_chunks          # chunk rows
        assert self.cr <= 32767, "gather idx must fit int16"
        self.own_pad = -(-self.own // self.proj_tile) * self.proj_tile
        self.wv_rows = self.own_pad + 128  # spare rows for the padding sink
        assert self.wv_rows <= 32767
        self.dummy_row = self.own_pad  # scatter target for padding edges


def build_program(cfg, g_pad):
    """One SPMD program; per-core behavior differs only through input data."""
    nc = bacc.Bacc("TRN2", target_bir_lowering=False, debug=False)
    W = g_pad // 16

    xt = nc.dram_tensor("xt", [cfg.in_dim, cfg.np_pad], F32, kind="ExternalInput")
    xt_own = nc.dram_tensor("xt_own", [cfg.in_dim, cfg.own_pad], F32, kind="ExternalInput")
    w_kv = nc.dram_tensor("w_kv", [cfg.in_dim, 2 * cfg.hid], F32, kind="ExternalInput")
    w_q = nc.dram_tensor("w_q", [cfg.in_dim, cfg.hid], F32, kind="ExternalInput")
    b_kv = nc.dram_tensor("b_kv", [128, 2 * cfg.hid], F32, kind="ExternalInput")
    b_q = nc.dram_tensor("b_q", [128, cfg.hid], F32, kind="ExternalInput")
    kv_idx = nc.dram_tensor("kv_idx", [cfg.n_chunks, 128, W], I16, kind="ExternalInput")
    q_idx = nc.dram_tensor("q_idx", [cfg.n_chunks, 128, W], I16, kind="ExternalInput")
    sc_idx = nc.dram_tensor("sc_idx", [cfg.n_chunks, 128, W], I16, kind="ExternalInput")

    wv = nc.dram_tensor("wv", [cfg.wv_rows, cfg.hid], F32, kind="ExternalOutput")

    KVDT = mybir.dt.bfloat16 if cfg.kv_bf16 else F32
    kv_tab = nc.dram_tensor("kv_tab", [cfg.np_pad, 2 * cfg.hid], KVDT)
    q_tab = nc.dram_tensor("q_tab", [cfg.own_pad, cfg.hid], F32)

    PT = cfg.proj_tile
    B = cfg.batch
    BC = B // 128  # column groups per batch tile

    with tile.TileContext(nc) as tc:
        with (
            tc.tile_pool(name="const", bufs=1) as cpool,
            tc.tile_pool(name="proj", bufs=3) as ppool,
            tc.tile_pool(name="psum", bufs=4, space="PSUM") as psum,
            tc.tile_pool(name="edge", bufs=3) as epool,
            tc.tile_pool(name="idx", bufs=3) as ipool,
        ):
            w_kv_t = cpool.tile([cfg.in_dim, 2 * cfg.hid], F32)
            w_q_t = cpool.tile([cfg.in_dim, cfg.hid], F32)
            b_kv_t = cpool.tile([128, 2 * cfg.hid], F32)
            b_q_t = cpool.tile([128, cfg.hid], F32)
            nc.sync.dma_start(w_kv_t[:], w_kv[:])
            nc.sync.dma_start(w_q_t[:], w_q[:])
            nc.sync.dma_start(b_kv_t[:], b_kv[:])
            nc.sync.dma_start(b_q_t[:], b_q[:])

            zt = cpool.tile([128, 4 * cfg.hid], F32)
            nc.vector.memset(zt[:], 0.0)
            for r in range(0, cfg.wv_rows, 512):
                rows = min(512, cfg.wv_rows - r)
                zview = wv[r:r + rows, :].rearrange("(s p) e -> p s e", p=128)
                nc.sync.dma_start(
                    zview, zt[:, :rows].rearrange("p (s e) -> p s e", e=cfg.hid))

            def project(src_dram, n_pad, w_t, b_t, out_dram, out_width, odt=F32):
                for g in range(n_pad // PT):
                    xt_t = ppool.tile([128, PT], F32, tag="xt_t")
                    nc.sync.dma_start(xt_t[:], src_dram[:, g * PT:(g + 1) * PT])
                    out_sb = ppool.tile([128, PT // 128, out_width], odt, tag="out_sb")
                    for s in range(PT // 128):
                        ps = psum.tile([128, out_width], F32)
                        nc.tensor.matmul(
                            ps[:], xt_t[:, s * 128:(s + 1) * 128], w_t[:],
                            start=True, stop=True,
                        )
                        nc.vector.tensor_add(out_sb[:, s, :], ps[:], b_t[:])
                    dview = out_dram[g * PT:(g + 1) * PT, :].rearrange(
                        "(s p) e -> p s e", p=128)
                    nc.sync.dma_start(dview, out_sb[:])

            project(xt, cfg.np_pad, w_kv_t, b_kv_t, kv_tab, 2 * cfg.hid, KVDT)
            project(xt_own, cfg.own_pad, w_q_t, b_q_t, q_tab, cfg.hid)

            for ch in range(cfg.n_chunks):
                kv_src = kv_tab[ch * cfg.cr:(ch + 1) * cfg.cr, :]
                for b in range(g_pad // B):
                    c0 = b * (B // 16)
                    kvi = ipool.tile([128, B // 16], I16, tag="kvi")
                    qi = ipool.tile([128, B // 16], I16, tag="qi")
                    sci = ipool.tile([128, B // 16], I16, tag="sci")
                    nc.sync.dma_start(kvi[:], kv_idx[ch, :, c0:c0 + B // 16])
                    nc.sync.dma_start(qi[:], q_idx[ch, :, c0:c0 + B // 16])
                    nc.sync.dma_start(sci[:], sc_idx[ch, :, c0:c0 + B // 16])

                    kv_t = epool.tile([128, BC, 2 * cfg.hid], KVDT, tag="kv_t")
                    q_t = epool.tile([128, BC, cfg.hid], F32, tag="q_t")
                    nc.gpsimd.dma_gather(
                        kv_t[:], kv_src, kvi[:], B, B, 2 * cfg.hid)
                    nc.gpsimd.dma_gather(
                        q_t[:], q_tab[:], qi[:], B, B, cfg.hid)

                    prod = epool.tile([128, BC, cfg.hid], F32, tag="prod")
                    nc.vector.tensor_mul(prod[:], kv_t[:, :, :cfg.hid], q_t[:])

                    sc = epool.tile([128, BC, cfg.heads], F32, tag="sc")
                    nc.vector.reduce_sum(
                        sc[:],
                        prod[:].rearrange("p c (h d) -> p c h d", d=cfg.hdim),
                        axis=mybir.AxisListType.X,
                    )
                    # clip(dot/scale, ±5) == clip(dot, ±5*scale) then /scale
                    lim = 5.0 * float(np.sqrt(cfg.hdim))
                    nc.vector.tensor_scalar_min(sc[:], sc[:], lim)
                    nc.vector.tensor_scalar_max(sc[:], sc[:], -lim)
                    ex = epool.tile([128, BC, cfg.heads], F32, tag="ex")
                    nc.scalar.activation(
                        ex[:], sc[:], mybir.ActivationFunctionType.Exp,
                        scale=float(1.0 / np.sqrt(cfg.hdim)),
                    )

                    msg = epool.tile([128, BC, cfg.hid], F32, tag="msg")
                    nc.vector.tensor_mul(
                        msg[:].rearrange("p c (h d) -> p c h d", d=cfg.hdim),
                        kv_t[:, :, cfg.hid:].rearrange(
                            "p c (h d) -> p c h d", d=cfg.hdim),
                        ex[:].unsqueeze(-1).broadcast_to(
                            [128, BC, cfg.heads, cfg.hdim]),
                    )
                    nc.gpsimd.dma_scatter_add(
                        wv[:], msg[:], sci[:], B, B, cfg.hid)
    nc.finalize()
    return nc


def _wrap16(a, g_pad):
    """[n] -> [128, g_pad//16] int16: idx i at [i%16 (+16k replicas), i//16]."""
    w = a.reshape(g_pad // 16, 16).T.astype(np.int16)  # [16, W]
    return np.tile(w, (8, 1))


def _schedule_batches(dst_local, batch):
    """Assign edges to batches of size `batch` so that no dst row appears
    twice within one batch (dma_scatter_add RMW races on duplicate rows
    within a single call). Returns (n_batches, edge order as an index
    array grouped by batch, per-batch counts)."""
    cnt = len(dst_local)
    if cnt == 0:
        return 1, np.empty(0, np.int64), np.zeros(1, np.int64)
    order = np.argsort(dst_local, kind="stable")
    uniq, starts, degs = np.unique(
        dst_local[order], return_index=True, return_counts=True)
    nb = max(-(-cnt // batch), int(degs.max()))
    big_first = np.argsort(-degs, kind="stable")
    while True:
        fills = np.zeros(nb, np.int64)
        bin_of = np.empty(cnt, np.int64)
        ok = True
        for gi in big_first:
            d = degs[gi]
            cand = np.argsort(fills, kind="stable")[:d]
            if fills[cand[-1]] >= batch:
                ok = False
                break
            fills[cand] += 1
            s = starts[gi]
            bin_of[order[s:s + d]] = cand
        if ok:
            break
        nb += 1
    batch_order = np.argsort(bin_of, kind="stable")
    counts = np.bincount(bin_of, minlength=nb)
    return nb, batch_order, counts


def prepare_inputs(cfg, x, src, dst, Wq, bq, Wk, bk, Wv, bv):
    x = np.asarray(x, np.float32)
    src = np.asarray(src, np.int64)
    dst = np.asarray(dst, np.int64)

    xt = np.zeros((cfg.in_dim, cfg.np_pad), np.float32)
    xt[:, :cfg.n_nodes] = x.T
    w_kv = np.concatenate([np.asarray(Wk, np.float32),
                           np.asarray(Wv, np.float32)], axis=1)
    b_kv = np.tile(np.concatenate([np.asarray(bk, np.float32),
                                   np.asarray(bv, np.float32)])[None, :], (128, 1))
    w_q = np.asarray(Wq, np.float32)
    b_q = np.tile(np.asarray(bq, np.float32)[None, :], (128, 1))

    core_of = dst // cfg.own
    chunk_of = src // cfg.cr

    # per-(core, chunk) edge lists, scheduled into duplicate-free batches
    groups = {}
    nb_max = 1
    for c in range(cfg.n_cores):
        in_c = np.nonzero(core_of == c)[0]
        ch_c = chunk_of[in_c]
        for ch in range(cfg.n_chunks):
            e = in_c[ch_c == ch]
            nb, border, counts = _schedule_batches(
                (dst[e] - c * cfg.own), cfg.batch)
            groups[(c, ch)] = (e[border] if len(e) else e, counts)
            nb_max = max(nb_max, nb)
    g_pad = nb_max * cfg.batch

    in_maps = []
    for c in range(cfg.n_cores):
        kvi = np.zeros((cfg.n_chunks, 128, g_pad // 16), np.int16)
        qi = np.zeros((cfg.n_chunks, 128, g_pad // 16), np.int16)
        sci = np.zeros((cfg.n_chunks, 128, g_pad // 16), np.int16)
        for ch in range(cfg.n_chunks):
            e, counts = groups[(c, ch)]
            kv_l = np.zeros(g_pad, np.int64)
            q_l = np.zeros(g_pad, np.int64)
            sc_l = np.full(g_pad, cfg.dummy_row, np.int64)
            pos = 0
            off = 0
            for b, cnt in enumerate(counts):
                eb = e[pos:pos + cnt]
                kv_l[off:off + cnt] = src[eb] - ch * cfg.cr
                q_l[off:off + cnt] = dst[eb] - c * cfg.own
                sc_l[off:off + cnt] = dst[eb] - c * cfg.own
                pos += cnt
                off += cfg.batch
            kvi[ch] = _wrap16(kv_l, g_pad)
            qi[ch] = _wrap16(q_l, g_pad)
            sci[ch] = _wrap16(sc_l, g_pad)

        xt_own = np.zeros((cfg.in_dim, cfg.own_pad), np.float32)
        xt_own[:, :cfg.own] = x[c * cfg.own:(c + 1) * cfg.own].T
        in_maps.append({
            "xt": xt, "xt_own": xt_own,
            "w_kv": w_kv, "w_q": w_q, "b_kv": b_kv, "b_q": b_q,
            "kv_idx": kvi, "q_idx": qi, "sc_idx": sci,
        })
    return in_maps, g_pad


def kernel(x, src, dst, Wq, bq, Wk, bk, Wv, bv):
    cfg = Cfg()
    in_maps, g_pad = prepare_inputs(cfg, x, src, dst, Wq, bq, Wk, bk, Wv, bv)
    nc = build_program(cfg, g_pad)
    res = run_bass_kernel_spmd(nc, in_maps, list(range(cfg.n_cores)))
    out = np.concatenate(
        [res.results[c]["wv"][:cfg.own] for c in range(cfg.n_cores)], axis=0)
    return out.reshape(cfg.n_nodes, cfg.heads, cfg.hdim)



# revision 2
# speedup vs baseline: 1.0791x; 1.0791x over previous
"""Multi-head graph attention message passing on 8 Trainium2 cores.

Strategy (graph/data parallel, per the dst-sharding scheme):
  - Nodes sharded by dst across 8 cores (12500 each). Each core owns the
    wV rows for its dst range; segment_sum is local via hardware
    dma_scatter_add (CCE accumulate in the SDMA datapath).
  - Q/K/V projections: small weights replicated; every core computes the
    full K,V tables (replicated compute, no cross-core traffic) and the
    Q table for its own node range only. K,V stored interleaved per node
    row [K|V] so one dma_gather per edge fetches both.
  - Edges routed by dst partition on host; within a core, grouped by src
    chunk (4 chunks) so dma_gather int16 indices stay in range.
"""

import numpy as np

import concourse.bacc as bacc
import concourse.mybir as mybir
import concourse.tile as tile
from concourse.bass_utils import run_bass_kernel_spmd

F32 = mybir.dt.float32
I16 = mybir.dt.int16


class Cfg:
    n_nodes = 100000
    n_edges = 1600000
    in_dim = 128
    heads = 8
    hdim = 16
    hid = 128          # heads * hdim
    n_cores = 8
    n_chunks = 4       # src chunks for int16 gather indices
    batch = 1024       # edges per device batch (dma_gather caps near 1024 idxs/call)
    proj_tile = 512    # nodes per projection DMA group
    kv_bf16 = False    # store K,V tables in bf16 (halves gather traffic)

    def __init__(self, **kw):
        for k, v in kw.items():
            setattr(self, k, v)
        assert self.n_nodes % self.n_cores == 0
        self.own = self.n_nodes // self.n_cores
        # padded full node count: multiple of proj_tile and n_chunks
        m = self.proj_tile * self.n_chunks
        self.np_pad = -(-self.n_nodes // m) * m
        self.cr = self.np_pad // self.n_chunks          # chunk rows
        assert self.cr <= 32767, "gather idx must fit int16"
        self.own_pad = -(-self.own // self.proj_tile) * self.proj_tile
        self.wv_rows = self.own_pad + 128  # spare rows for the padding sink
        assert self.wv_rows <= 32767
        self.dummy_row = self.own_pad  # scatter target for padding edges


def build_program(cfg, g_pad):
    """One SPMD program; per-core behavior differs only through input data."""
    nc = bacc.Bacc("TRN2", target_bir_lowering=False, debug=False)
    W = g_pad // 16

    xt = nc.dram_tensor("xt", [cfg.in_dim, cfg.np_pad], F32, kind="ExternalInput")
    xt_own = nc.dram_tensor("xt_own", [cfg.in_dim, cfg.own_pad], F32, kind="ExternalInput")
    w_kv = nc.dram_tensor("w_kv", [cfg.in_dim, 2 * cfg.hid], F32, kind="ExternalInput")
    w_q = nc.dram_tensor("w_q", [cfg.in_dim, cfg.hid], F32, kind="ExternalInput")
    b_kv = nc.dram_tensor("b_kv", [128, 2 * cfg.hid], F32, kind="ExternalInput")
    b_q = nc.dram_tensor("b_q", [128, cfg.hid], F32, kind="ExternalInput")
    kv_idx = nc.dram_tensor("kv_idx", [cfg.n_chunks, 128, W], I16, kind="ExternalInput")
    q_idx = nc.dram_tensor("q_idx", [cfg.n_chunks, 128, W], I16, kind="ExternalInput")
    sc_idx = nc.dram_tensor("sc_idx", [cfg.n_chunks, 128, W], I16, kind="ExternalInput")

    wv = nc.dram_tensor("wv", [cfg.wv_rows, cfg.hid], F32, kind="ExternalOutput")

    KVDT = mybir.dt.bfloat16 if cfg.kv_bf16 else F32
    kv_tab = nc.dram_tensor("kv_tab", [cfg.np_pad, 2 * cfg.hid], KVDT)
    q_tab = nc.dram_tensor("q_tab", [cfg.own_pad, cfg.hid], F32)

    PT = cfg.proj_tile
    B = cfg.batch
    BC = B // 128  # column groups per batch tile

    with tile.TileContext(nc) as tc:
        with (
            tc.tile_pool(name="const", bufs=1) as cpool,
            tc.tile_pool(name="proj", bufs=3) as ppool,
            tc.tile_pool(name="psum", bufs=4, space="PSUM") as psum,
            tc.tile_pool(name="edge", bufs=4) as epool,
            tc.tile_pool(name="idx", bufs=3) as ipool,
        ):
            w_kv_t = cpool.tile([cfg.in_dim, 2 * cfg.hid], F32)
            w_q_t = cpool.tile([cfg.in_dim, cfg.hid], F32)
            b_kv_t = cpool.tile([128, 2 * cfg.hid], F32)
            b_q_t = cpool.tile([128, cfg.hid], F32)
            nc.sync.dma_start(w_kv_t[:], w_kv[:])
            nc.sync.dma_start(w_q_t[:], w_q[:])
            nc.sync.dma_start(b_kv_t[:], b_kv[:])
            nc.sync.dma_start(b_q_t[:], b_q[:])

            zt = cpool.tile([128, 4 * cfg.hid], F32)
            nc.vector.memset(zt[:], 0.0)
            for r in range(0, cfg.wv_rows, 512):
                rows = min(512, cfg.wv_rows - r)
                zview = wv[r:r + rows, :].rearrange("(s p) e -> p s e", p=128)
                nc.sync.dma_start(
                    zview, zt[:, :rows].rearrange("p (s e) -> p s e", e=cfg.hid))

            def project(src_dram, n_pad, w_t, b_t, out_dram, out_width, odt=F32):
                for g in range(n_pad // PT):
                    xt_t = ppool.tile([128, PT], F32, tag="xt_t")
                    nc.sync.dma_start(xt_t[:], src_dram[:, g * PT:(g + 1) * PT])
                    out_sb = ppool.tile([128, PT // 128, out_width], odt, tag="out_sb")
                    for s in range(PT // 128):
                        ps = psum.tile([128, out_width], F32)
                        nc.tensor.matmul(
                            ps[:], xt_t[:, s * 128:(s + 1) * 128], w_t[:],
                            start=True, stop=True,
                        )
                        nc.vector.tensor_add(out_sb[:, s, :], ps[:], b_t[:])
                    dview = out_dram[g * PT:(g + 1) * PT, :].rearrange(
                        "(s p) e -> p s e", p=128)
                    nc.sync.dma_start(dview, out_sb[:])

            project(xt, cfg.np_pad, w_kv_t, b_kv_t, kv_tab, 2 * cfg.hid, KVDT)
            project(xt_own, cfg.own_pad, w_q_t, b_q_t, q_tab, cfg.hid)

            for ch in range(cfg.n_chunks):
                kv_src = kv_tab[ch * cfg.cr:(ch + 1) * cfg.cr, :]
                for b in range(g_pad // B):
                    c0 = b * (B // 16)
                    kvi = ipool.tile([128, B // 16], I16, tag="kvi")
                    qi = ipool.tile([128, B // 16], I16, tag="qi")
                    sci = ipool.tile([128, B // 16], I16, tag="sci")
                    nc.sync.dma_start(kvi[:], kv_idx[ch, :, c0:c0 + B // 16])
                    nc.sync.dma_start(qi[:], q_idx[ch, :, c0:c0 + B // 16])
                    nc.sync.dma_start(sci[:], sc_idx[ch, :, c0:c0 + B // 16])

                    kv_t = epool.tile([128, BC, 2 * cfg.hid], KVDT, tag="kv_t")
                    q_t = epool.tile([128, BC, cfg.hid], F32, tag="q_t")
                    nc.gpsimd.dma_gather(
                        kv_t[:], kv_src, kvi[:], B, B, 2 * cfg.hid)
                    nc.gpsimd.dma_gather(
                        q_t[:], q_tab[:], qi[:], B, B, cfg.hid)

                    prod = epool.tile([128, BC, cfg.hid], F32, tag="prod")
                    nc.vector.tensor_mul(prod[:], kv_t[:, :, :cfg.hid], q_t[:])

                    sc = epool.tile([128, BC, cfg.heads], F32, tag="sc")
                    nc.vector.reduce_sum(
                        sc[:],
                        prod[:].rearrange("p c (h d) -> p c h d", d=cfg.hdim),
                        axis=mybir.AxisListType.X,
                    )
                    # clip(dot/scale, ±5) == clip(dot, ±5*scale) then /scale
                    lim = 5.0 * float(np.sqrt(cfg.hdim))
                    nc.vector.tensor_scalar_min(sc[:], sc[:], lim)
                    nc.vector.tensor_scalar_max(sc[:], sc[:], -lim)
                    ex = epool.tile([128, BC, cfg.heads], F32, tag="ex")
                    nc.scalar.activation(
                        ex[:], sc[:], mybir.ActivationFunctionType.Exp,
                        scale=float(1.0 / np.sqrt(cfg.hdim)),
                    )

                    msg = epool.tile([128, BC, cfg.hid], F32, tag="msg")
                    nc.vector.tensor_mul(
                        msg[:].rearrange("p c (h d) -> p c h d", d=cfg.hdim),
                        kv_t[:, :, cfg.hid:].rearrange(
                            "p c (h d) -> p c h d", d=cfg.hdim),
                        ex[:].unsqueeze(-1).broadcast_to(
                            [128, BC, cfg.heads, cfg.hdim]),
                    )
                    nc.gpsimd.dma_scatter_add(
                        wv[:], msg[:], sci[:], B, B, cfg.hid)
    nc.finalize()
    return nc


def _wrap16(a, g_pad):
    """[n] -> [128, g_pad//16] int16: idx i at [i%16 (+16k replicas), i//16]."""
    w = a.reshape(g_pad // 16, 16).T.astype(np.int16)  # [16, W]
    return np.tile(w, (8, 1))


def _schedule_batches(dst_local, batch):
    """Assign edges to batches of size `batch` so that no dst row appears
    twice within one batch (dma_scatter_add RMW races on duplicate rows
    within a single call). Returns (n_batches, edge order as an index
    array grouped by batch, per-batch counts)."""
    cnt = len(dst_local)
    if cnt == 0:
        return 1, np.empty(0, np.int64), np.zeros(1, np.int64)
    order = np.argsort(dst_local, kind="stable")
    uniq, starts, degs = np.unique(
        dst_local[order], return_index=True, return_counts=True)
    nb = max(-(-cnt // batch), int(degs.max()))
    big_first = np.argsort(-degs, kind="stable")
    while True:
        fills = np.zeros(nb, np.int64)
        bin_of = np.empty(cnt, np.int64)
        ok = True
        for gi in big_first:
            d = degs[gi]
            cand = np.argsort(fills, kind="stable")[:d]
            if fills[cand[-1]] >= batch:
                ok = False
                break
            fills[cand] += 1
            s = starts[gi]
            bin_of[order[s:s + d]] = cand
        if ok:
            break
        nb += 1
    batch_order = np.argsort(bin_of, kind="stable")
    counts = np.bincount(bin_of, minlength=nb)
    return nb, batch_order, counts


def prepare_inputs(cfg, x, src, dst, Wq, bq, Wk, bk, Wv, bv):
    x = np.asarray(x, np.float32)
    src = np.asarray(src, np.int64)
    dst = np.asarray(dst, np.int64)

    xt = np.zeros((cfg.in_dim, cfg.np_pad), np.float32)
    xt[:, :cfg.n_nodes] = x.T
    w_kv = np.concatenate([np.asarray(Wk, np.float32),
                           np.asarray(Wv, np.float32)], axis=1)
    b_kv = np.tile(np.concatenate([np.asarray(bk, np.float32),
                                   np.asarray(bv, np.float32)])[None, :], (128, 1))
    w_q = np.asarray(Wq, np.float32)
    b_q = np.tile(np.asarray(bq, np.float32)[None, :], (128, 1))

    core_of = dst // cfg.own
    chunk_of = src // cfg.cr

    # per-(core, chunk) edge lists, scheduled into duplicate-free batches
    groups = {}
    nb_max = 1
    for c in range(cfg.n_cores):
        in_c = np.nonzero(core_of == c)[0]
        ch_c = chunk_of[in_c]
        for ch in range(cfg.n_chunks):
            e = in_c[ch_c == ch]
            nb, border, counts = _schedule_batches(
                (dst[e] - c * cfg.own), cfg.batch)
            groups[(c, ch)] = (e[border] if len(e) else e, counts)
            nb_max = max(nb_max, nb)
    g_pad = nb_max * cfg.batch

    in_maps = []
    for c in range(cfg.n_cores):
        kvi = np.zeros((cfg.n_chunks, 128, g_pad // 16), np.int16)
        qi = np.zeros((cfg.n_chunks, 128, g_pad // 16), np.int16)
        sci = np.zeros((cfg.n_chunks, 128, g_pad // 16), np.int16)
        for ch in range(cfg.n_chunks):
            e, counts = groups[(c, ch)]
            kv_l = np.zeros(g_pad, np.int64)
            q_l = np.zeros(g_pad, np.int64)
            sc_l = np.full(g_pad, cfg.dummy_row, np.int64)
            pos = 0
            off = 0
            for b, cnt in enumerate(counts):
                eb = e[pos:pos + cnt]
                kv_l[off:off + cnt] = src[eb] - ch * cfg.cr
                q_l[off:off + cnt] = dst[eb] - c * cfg.own
                sc_l[off:off + cnt] = dst[eb] - c * cfg.own
                pos += cnt
                off += cfg.batch
            kvi[ch] = _wrap16(kv_l, g_pad)
            qi[ch] = _wrap16(q_l, g_pad)
            sci[ch] = _wrap16(sc_l, g_pad)

        xt_own = np.zeros((cfg.in_dim, cfg.own_pad), np.float32)
        xt_own[:, :cfg.own] = x[c * cfg.own:(c + 1) * cfg.own].T
        in_maps.append({
            "xt": xt, "xt_own": xt_own,
            "w_kv": w_kv, "w_q": w_q, "b_kv": b_kv, "b_q": b_q,
            "kv_idx": kvi, "q_idx": qi, "sc_idx": sci,
        })
    return in_maps, g_pad


def kernel(x, src, dst, Wq, bq, Wk, bk, Wv, bv):
    cfg = Cfg()
    in_maps, g_pad = prepare_inputs(cfg, x, src, dst, Wq, bq, Wk, bk, Wv, bv)
    nc = build_program(cfg, g_pad)
    res = run_bass_kernel_spmd(nc, in_maps, list(range(cfg.n_cores)))
    out = np.concatenate(
        [res.results[c]["wv"][:cfg.own] for c in range(cfg.n_cores)], axis=0)
    return out.reshape(cfg.n_nodes, cfg.heads, cfg.hdim)



# revision 3
# speedup vs baseline: 1.1434x; 1.0597x over previous
"""Multi-head graph attention message passing on 8 Trainium2 cores.

Strategy (graph/data parallel, per the dst-sharding scheme):
  - Nodes sharded by dst across 8 cores (12500 each). Each core owns the
    wV rows for its dst range; segment_sum is local via hardware
    dma_scatter_add (CCE accumulate in the SDMA datapath).
  - Q/K/V projections: small weights replicated; every core computes the
    full K,V tables (replicated compute, no cross-core traffic) and the
    Q table for its own node range only. K,V stored interleaved per node
    row [K|V] so one dma_gather per edge fetches both.
  - Edges routed by dst partition on host; within a core, grouped by src
    chunk (4 chunks) so dma_gather int16 indices stay in range.
"""

import numpy as np

import concourse.bacc as bacc
import concourse.mybir as mybir
import concourse.tile as tile
from concourse.bass_utils import run_bass_kernel_spmd

F32 = mybir.dt.float32
I16 = mybir.dt.int16


class Cfg:
    n_nodes = 100000
    n_edges = 1600000
    in_dim = 128
    heads = 8
    hdim = 16
    hid = 128          # heads * hdim
    n_cores = 8
    n_chunks = 4       # src chunks for int16 gather indices
    batch = 1024       # edges per device batch (dma_gather caps near 1024 idxs/call)
    proj_tile = 512    # nodes per projection DMA group
    kv_bf16 = False    # store K,V tables in bf16 (halves gather traffic)

    def __init__(self, **kw):
        for k, v in kw.items():
            setattr(self, k, v)
        assert self.n_nodes % self.n_cores == 0
        self.own = self.n_nodes // self.n_cores
        # padded full node count: multiple of proj_tile and n_chunks
        m = self.proj_tile * self.n_chunks
        self.np_pad = -(-self.n_nodes // m) * m
        self.cr = self.np_pad // self.n_chunks          # chunk rows
        assert self.cr <= 32767, "gather idx must fit int16"
        self.own_pad = -(-self.own // self.proj_tile) * self.proj_tile
        self.wv_rows = self.own_pad + 128  # spare rows for the padding sink
        assert self.wv_rows <= 32767
        self.dummy_row = self.own_pad  # scatter target for padding edges


def build_program(cfg, g_pad):
    """One SPMD program; per-core behavior differs only through input data."""
    nc = bacc.Bacc("TRN2", target_bir_lowering=False, debug=False)
    W = g_pad // 16

    xt = nc.dram_tensor("xt", [cfg.in_dim, cfg.np_pad], F32, kind="ExternalInput")
    xt_own = nc.dram_tensor("xt_own", [cfg.in_dim, cfg.own_pad], F32, kind="ExternalInput")
    w_kv = nc.dram_tensor("w_kv", [cfg.in_dim, 2 * cfg.hid], F32, kind="ExternalInput")
    w_q = nc.dram_tensor("w_q", [cfg.in_dim, cfg.hid], F32, kind="ExternalInput")
    b_kv = nc.dram_tensor("b_kv", [128, 2 * cfg.hid], F32, kind="ExternalInput")
    b_q = nc.dram_tensor("b_q", [128, cfg.hid], F32, kind="ExternalInput")
    kv_idx = nc.dram_tensor("kv_idx", [cfg.n_chunks, 128, W], I16, kind="ExternalInput")
    q_idx = nc.dram_tensor("q_idx", [cfg.n_chunks, 128, W], I16, kind="ExternalInput")
    sc_idx = nc.dram_tensor("sc_idx", [cfg.n_chunks, 128, W], I16, kind="ExternalInput")

    wv = nc.dram_tensor("wv", [cfg.wv_rows, cfg.hid], F32, kind="ExternalOutput")

    KVDT = mybir.dt.bfloat16 if cfg.kv_bf16 else F32
    kv_tab = nc.dram_tensor("kv_tab", [cfg.np_pad, 2 * cfg.hid], KVDT)
    q_tab = nc.dram_tensor("q_tab", [cfg.own_pad, cfg.hid], F32)

    PT = cfg.proj_tile
    B = cfg.batch
    BC = B // 128  # column groups per batch tile

    with tile.TileContext(nc) as tc:
        with (
            tc.tile_pool(name="const", bufs=1) as cpool,
            tc.tile_pool(name="proj", bufs=3) as ppool,
            tc.tile_pool(name="psum", bufs=4, space="PSUM") as psum,
            tc.tile_pool(name="edge", bufs=6) as epool,
            tc.tile_pool(name="idx", bufs=3) as ipool,
        ):
            w_kv_t = cpool.tile([cfg.in_dim, 2 * cfg.hid], F32)
            w_q_t = cpool.tile([cfg.in_dim, cfg.hid], F32)
            b_kv_t = cpool.tile([128, 2 * cfg.hid], F32)
            b_q_t = cpool.tile([128, cfg.hid], F32)
            nc.sync.dma_start(w_kv_t[:], w_kv[:])
            nc.sync.dma_start(w_q_t[:], w_q[:])
            nc.sync.dma_start(b_kv_t[:], b_kv[:])
            nc.sync.dma_start(b_q_t[:], b_q[:])

            zt = cpool.tile([128, 4 * cfg.hid], F32)
            nc.vector.memset(zt[:], 0.0)
            for r in range(0, cfg.wv_rows, 512):
                rows = min(512, cfg.wv_rows - r)
                zview = wv[r:r + rows, :].rearrange("(s p) e -> p s e", p=128)
                nc.sync.dma_start(
                    zview, zt[:, :rows].rearrange("p (s e) -> p s e", e=cfg.hid))

            def project(src_dram, n_pad, w_t, b_t, out_dram, out_width, odt=F32):
                for g in range(n_pad // PT):
                    xt_t = ppool.tile([128, PT], F32, tag="xt_t")
                    nc.sync.dma_start(xt_t[:], src_dram[:, g * PT:(g + 1) * PT])
                    out_sb = ppool.tile([128, PT // 128, out_width], odt, tag="out_sb")
                    for s in range(PT // 128):
                        ps = psum.tile([128, out_width], F32)
                        nc.tensor.matmul(
                            ps[:], xt_t[:, s * 128:(s + 1) * 128], w_t[:],
                            start=True, stop=True,
                        )
                        nc.vector.tensor_add(out_sb[:, s, :], ps[:], b_t[:])
                    dview = out_dram[g * PT:(g + 1) * PT, :].rearrange(
                        "(s p) e -> p s e", p=128)
                    nc.sync.dma_start(dview, out_sb[:])

            project(xt, cfg.np_pad, w_kv_t, b_kv_t, kv_tab, 2 * cfg.hid, KVDT)
            project(xt_own, cfg.own_pad, w_q_t, b_q_t, q_tab, cfg.hid)

            for ch in range(cfg.n_chunks):
                kv_src = kv_tab[ch * cfg.cr:(ch + 1) * cfg.cr, :]
                for b in range(g_pad // B):
                    c0 = b * (B // 16)
                    kvi = ipool.tile([128, B // 16], I16, tag="kvi")
                    qi = ipool.tile([128, B // 16], I16, tag="qi")
                    sci = ipool.tile([128, B // 16], I16, tag="sci")
                    nc.sync.dma_start(kvi[:], kv_idx[ch, :, c0:c0 + B // 16])
                    nc.sync.dma_start(qi[:], q_idx[ch, :, c0:c0 + B // 16])
                    nc.sync.dma_start(sci[:], sc_idx[ch, :, c0:c0 + B // 16])

                    kv_t = epool.tile([128, BC, 2 * cfg.hid], KVDT, tag="kv_t")
                    q_t = epool.tile([128, BC, cfg.hid], F32, tag="q_t")
                    nc.gpsimd.dma_gather(
                        kv_t[:], kv_src, kvi[:], B, B, 2 * cfg.hid)
                    nc.gpsimd.dma_gather(
                        q_t[:], q_tab[:], qi[:], B, B, cfg.hid)

                    prod = epool.tile([128, BC, cfg.hid], F32, tag="prod")
                    nc.vector.tensor_mul(prod[:], kv_t[:, :, :cfg.hid], q_t[:])

                    sc = epool.tile([128, BC, cfg.heads], F32, tag="sc")
                    nc.vector.reduce_sum(
                        sc[:],
                        prod[:].rearrange("p c (h d) -> p c h d", d=cfg.hdim),
                        axis=mybir.AxisListType.X,
                    )
                    # clip(dot/scale, ±5) == clip(dot, ±5*scale) then /scale
                    lim = 5.0 * float(np.sqrt(cfg.hdim))
                    nc.vector.tensor_scalar_min(sc[:], sc[:], lim)
                    nc.vector.tensor_scalar_max(sc[:], sc[:], -lim)
                    ex = epool.tile([128, BC, cfg.heads], F32, tag="ex")
                    nc.scalar.activation(
                        ex[:], sc[:], mybir.ActivationFunctionType.Exp,
                        scale=float(1.0 / np.sqrt(cfg.hdim)),
                    )

                    msg = epool.tile([128, BC, cfg.hid], F32, tag="msg")
                    nc.vector.tensor_mul(
                        msg[:].rearrange("p c (h d) -> p c h d", d=cfg.hdim),
                        kv_t[:, :, cfg.hid:].rearrange(
                            "p c (h d) -> p c h d", d=cfg.hdim),
                        ex[:].unsqueeze(-1).broadcast_to(
                            [128, BC, cfg.heads, cfg.hdim]),
                    )
                    nc.gpsimd.dma_scatter_add(
                        wv[:], msg[:], sci[:], B, B, cfg.hid)
    nc.finalize()
    return nc


def _wrap16(a, g_pad):
    """[n] -> [128, g_pad//16] int16: idx i at [i%16 (+16k replicas), i//16]."""
    w = a.reshape(g_pad // 16, 16).T.astype(np.int16)  # [16, W]
    return np.tile(w, (8, 1))


def _schedule_batches(dst_local, batch):
    """Assign edges to batches of size `batch` so that no dst row appears
    twice within one batch (dma_scatter_add RMW races on duplicate rows
    within a single call). Returns (n_batches, edge order as an index
    array grouped by batch, per-batch counts)."""
    cnt = len(dst_local)
    if cnt == 0:
        return 1, np.empty(0, np.int64), np.zeros(1, np.int64)
    order = np.argsort(dst_local, kind="stable")
    uniq, starts, degs = np.unique(
        dst_local[order], return_index=True, return_counts=True)
    nb = max(-(-cnt // batch), int(degs.max()))
    big_first = np.argsort(-degs, kind="stable")
    while True:
        fills = np.zeros(nb, np.int64)
        bin_of = np.empty(cnt, np.int64)
        ok = True
        for gi in big_first:
            d = degs[gi]
            cand = np.argsort(fills, kind="stable")[:d]
            if fills[cand[-1]] >= batch:
                ok = False
                break
            fills[cand] += 1
            s = starts[gi]
            bin_of[order[s:s + d]] = cand
        if ok:
            break
        nb += 1
    batch_order = np.argsort(bin_of, kind="stable")
    counts = np.bincount(bin_of, minlength=nb)
    return nb, batch_order, counts


def prepare_inputs(cfg, x, src, dst, Wq, bq, Wk, bk, Wv, bv):
    x = np.asarray(x, np.float32)
    src = np.asarray(src, np.int64)
    dst = np.asarray(dst, np.int64)

    xt = np.zeros((cfg.in_dim, cfg.np_pad), np.float32)
    xt[:, :cfg.n_nodes] = x.T
    w_kv = np.concatenate([np.asarray(Wk, np.float32),
                           np.asarray(Wv, np.float32)], axis=1)
    b_kv = np.tile(np.concatenate([np.asarray(bk, np.float32),
                                   np.asarray(bv, np.float32)])[None, :], (128, 1))
    w_q = np.asarray(Wq, np.float32)
    b_q = np.tile(np.asarray(bq, np.float32)[None, :], (128, 1))

    core_of = dst // cfg.own
    chunk_of = src // cfg.cr

    # per-(core, chunk) edge lists, scheduled into duplicate-free batches
    groups = {}
    nb_max = 1
    for c in range(cfg.n_cores):
        in_c = np.nonzero(core_of == c)[0]
        ch_c = chunk_of[in_c]
        for ch in range(cfg.n_chunks):
            e = in_c[ch_c == ch]
            nb, border, counts = _schedule_batches(
                (dst[e] - c * cfg.own), cfg.batch)
            groups[(c, ch)] = (e[border] if len(e) else e, counts)
            nb_max = max(nb_max, nb)
    g_pad = nb_max * cfg.batch

    in_maps = []
    for c in range(cfg.n_cores):
        kvi = np.zeros((cfg.n_chunks, 128, g_pad // 16), np.int16)
        qi = np.zeros((cfg.n_chunks, 128, g_pad // 16), np.int16)
        sci = np.zeros((cfg.n_chunks, 128, g_pad // 16), np.int16)
        for ch in range(cfg.n_chunks):
            e, counts = groups[(c, ch)]
            kv_l = np.zeros(g_pad, np.int64)
            q_l = np.zeros(g_pad, np.int64)
            sc_l = np.full(g_pad, cfg.dummy_row, np.int64)
            pos = 0
            off = 0
            for b, cnt in enumerate(counts):
                eb = e[pos:pos + cnt]
                kv_l[off:off + cnt] = src[eb] - ch * cfg.cr
                q_l[off:off + cnt] = dst[eb] - c * cfg.own
                sc_l[off:off + cnt] = dst[eb] - c * cfg.own
                pos += cnt
                off += cfg.batch
            kvi[ch] = _wrap16(kv_l, g_pad)
            qi[ch] = _wrap16(q_l, g_pad)
            sci[ch] = _wrap16(sc_l, g_pad)

        xt_own = np.zeros((cfg.in_dim, cfg.own_pad), np.float32)
        xt_own[:, :cfg.own] = x[c * cfg.own:(c + 1) * cfg.own].T
        in_maps.append({
            "xt": xt, "xt_own": xt_own,
            "w_kv": w_kv, "w_q": w_q, "b_kv": b_kv, "b_q": b_q,
            "kv_idx": kvi, "q_idx": qi, "sc_idx": sci,
        })
    return in_maps, g_pad


def kernel(x, src, dst, Wq, bq, Wk, bk, Wv, bv):
    cfg = Cfg()
    in_maps, g_pad = prepare_inputs(cfg, x, src, dst, Wq, bq, Wk, bk, Wv, bv)
    nc = build_program(cfg, g_pad)
    res = run_bass_kernel_spmd(nc, in_maps, list(range(cfg.n_cores)))
    out = np.concatenate(
        [res.results[c]["wv"][:cfg.own] for c in range(cfg.n_cores)], axis=0)
    return out.reshape(cfg.n_nodes, cfg.heads, cfg.hdim)

